# revision 50
# baseline (speedup 1.0000x reference)
"""Trainium2 Bass kernel: causal multi-head self-attention block (B=8, T=1024, E=768, H=12).

Sharding: data-parallel over batch - one batch element per NeuronCore, 8 cores,
no collectives. Each core computes the full attention block for its batch row.

v2: host pre-transposes X and pre-casts X/W1/W2 to bf16 (no on-device casts or
PE transposes), drops the K-projection bias (softmax-invariant), single rotating
PSUM pool for scores+AV, interleaved accumulation chains for PSUM bank
alternation, batched ln/exp reciprocal + DMA partition-broadcast for the
softmax denominator (no GpSimd broadcast on the critical path).

Self-contained: hardcodes all shapes; only imports concourse (installed
system-wide) and ml_dtypes.
"""

import numpy as np
import ml_dtypes

B, T, E, H, Dh = 8, 1024, 768, 12, 64
F3 = 3 * E            # 2304
KC = E // 128         # 6 e-chunks
MT = T // 128         # 8 t-tiles
NPAIR = H // 2        # 6 head pairs
SCALE = 1.0 / float(np.sqrt(Dh))

_NC_CACHE = None


def build_nc():
    import concourse.mybir as mybir
    from concourse import bacc
    from concourse.tile import TileContext
    from concourse.masks import make_identity

    bf = mybir.dt.bfloat16
    f32 = mybir.dt.float32
    EXP = mybir.ActivationFunctionType.Exp
    LN = mybir.ActivationFunctionType.Ln
    COPY = mybir.ActivationFunctionType.Copy
    IDENT = mybir.ActivationFunctionType.Identity
    ACT_SET_LN_EXP = 6  # natural_log_exp_and_others: holds both Ln and Exp

    nc = bacc.Bacc("TRN2", target_bir_lowering=False, debug=False, num_devices=B, name="attn_dp2")

    XT_ext = nc.declare_dram_parameter("XT", [E, T], bf, isOutput=False)
    W1_ext = nc.declare_dram_parameter("W1", [E, F3], bf, isOutput=False)
    b1_ext = nc.declare_dram_parameter("b1", [F3], f32, isOutput=False)
    W2_ext = nc.declare_dram_parameter("W2", [E, E], bf, isOutput=False)
    b2_ext = nc.declare_dram_parameter("b2", [E], f32, isOutput=False)
    out_ext = nc.declare_dram_parameter("out", [T, E], f32, isOutput=True)

    with TileContext(nc) as tc:
        with (
            tc.tile_pool(name="persist", bufs=1) as persist,
            tc.tile_pool(name="stage", bufs=2) as stage,
            tc.tile_pool(name="ptpool", bufs=13) as ptpool,
            tc.tile_pool(name="zsbpool", bufs=3) as zsbpool,
            tc.tile_pool(name="dlnpool", bufs=2) as dlnpool,
            tc.tile_pool(name="recpool", bufs=2) as recpool,
            tc.tile_pool(name="bcpool", bufs=3) as bcpool,
            tc.tile_pool(name="opool", bufs=2) as opool,
            tc.tile_pool(name="att", bufs=3, space="PSUM") as att,
            tc.tile_pool(name="acc", bufs=2, space="PSUM") as acc,
        ):
            # ---- input DMAs first: the whole kernel is gated on W1/XT landing.
            # Interleaved per kc so the first qk chains can start as soon as the
            # first chunks arrive; W2 is DMA'd later (emitted inside the hp
            # loop) since it is only needed for the output projection.
            XT = persist.tile([128, KC, T], bf, tag="XT")
            W1bf = persist.tile([128, KC, F3], bf, tag="W1bf")
            W2bf = persist.tile([128, KC, E], bf, tag="W2bf")
            # Q/K weight columns + XT first (gate the first qk chains); V
            # columns of W1 are only needed by vproj, which runs later.
            # column-priority order: the first two qk tile pairs (m=0,1,6,7)
            # and the V block unblock early; remaining Q/K columns land last
            # (they are only needed from pair 2 on)
            for kc in range(KC):
                nc.sync.dma_start(out=XT[:, kc, :], in_=XT_ext[kc * 128:(kc + 1) * 128, :])
                nc.sync.dma_start(out=W1bf[:, kc, 0:256], in_=W1_ext[kc * 128:(kc + 1) * 128, 0:256])
                nc.sync.dma_start(out=W1bf[:, kc, 768:1024], in_=W1_ext[kc * 128:(kc + 1) * 128, 768:1024])
            for kc in range(KC):
                nc.sync.dma_start(out=W1bf[:, kc, 1536:2304], in_=W1_ext[kc * 128:(kc + 1) * 128, 1536:2304])
            for kc in range(KC):
                nc.sync.dma_start(out=W1bf[:, kc, 256:768], in_=W1_ext[kc * 128:(kc + 1) * 128, 256:768])
                nc.sync.dma_start(out=W1bf[:, kc, 1024:1536], in_=W1_ext[kc * 128:(kc + 1) * 128, 1024:1536])

            def w2_dma():
                for kc in range(KC):
                    nc.sync.dma_start(out=W2bf[:, kc, :], in_=W2_ext[kc * 128:(kc + 1) * 128, :])

            # One activation-table load for the whole kernel (covers Exp + Ln).
            nc.scalar.add_instruction(mybir.InstLoadActFuncSet(
                name=nc.get_next_instruction_name(), ins=[], outs=[],
                act_func_set_id=ACT_SET_LN_EXP))

            # identity for folding the outA partial into outproj_b's PE chain
            ident = persist.tile([128, 128], bf, tag="ident")
            make_identity(nc, ident[:])

            # ---- constants ----
            # multiplicative causal mask for the diagonal 128x128 block, for
            # both heads of a pair: mask[k, h01, q] = 1 where q >= k else 0
            diagmask = persist.tile([128, 2, 128], bf, tag="diagmask")
            nc.gpsimd.memset(diagmask[:], 1.0)
            for h01 in range(2):
                nc.gpsimd.affine_select(
                    out=diagmask[:, h01, :], in_=diagmask[:, h01, :],
                    compare_op=mybir.AluOpType.is_ge, fill=0.0, base=0,
                    pattern=[[1, 128]], channel_multiplier=-1,
                )

            # per-partition bias for the Q part of b1: b1qk[p, m] = b1[m*128 + p]
            # (K bias dropped: it shifts every score row by a per-query constant,
            # which softmax is invariant to.)
            b1qk = persist.tile([128, 6], f32, tag="b1qk")
            nc.sync.dma_start(
                out=b1qk[:], in_=b1_ext[0:768].rearrange("(m p) -> p m", p=128)
            )
            # row biases, pre-broadcast across partitions
            b1v_f = stage.tile([1, E], f32, tag="rowstage")
            nc.sync.dma_start(out=b1v_f[:], in_=b1_ext[None, 1536:2304])
            b1vb = persist.tile([128, E], f32, tag="b1vb")
            nc.gpsimd.partition_broadcast(b1vb[:], b1v_f[:])
            b2_f = stage.tile([1, E], f32, tag="rowstage")
            nc.sync.dma_start(out=b2_f[:], in_=b2_ext[None, :])
            b2b = persist.tile([128, E], f32, tag="b2b")
            nc.gpsimd.partition_broadcast(b2b[:], b2_f[:])

            # QK[p, m, t]: m 0..5 = Q^T blocks (f rows m*128..), m 6..11 = K^T blocks
            QK = persist.tile([128, 12, T], bf, tag="QK")

            def qk_mtile(m):
                # two 512-column chains, kc-interleaved so consecutive PE
                # writes alternate PSUM banks
                psA = acc.tile([128, 512], f32, tag="acc")
                psB = acc.tile([128, 512], f32, tag="acc")
                for kc in range(KC):
                    nc.tensor.matmul(
                        psA[:], W1bf[:, kc, m * 128:(m + 1) * 128],
                        XT[:, kc, 0:512], start=(kc == 0), stop=(kc == KC - 1))
                    nc.tensor.matmul(
                        psB[:], W1bf[:, kc, m * 128:(m + 1) * 128],
                        XT[:, kc, 512:1024], start=(kc == 0), stop=(kc == KC - 1))
                # evictions on DVE: ScalarE is the critical engine (exp) and
                # must not queue eviction work ahead of the exp stream
                if m < 6:
                    nc.vector.tensor_scalar_add(QK[:, m, 0:512], psA[:], b1qk[:, m:m + 1])
                    nc.vector.tensor_scalar_add(QK[:, m, 512:1024], psB[:], b1qk[:, m:m + 1])
                else:
                    nc.vector.tensor_copy(QK[:, m, 0:512], psA[:])
                    nc.vector.tensor_copy(QK[:, m, 512:1024], psB[:])

            # ---- V projection into V_aug[t-part, kt, h, 0:64] with ones col at 64 ----
            Vg = persist.tile([128, MT, H, Dh + 1], bf, tag="Vg")
            for mt in range(MT):
                nc.gpsimd.memset(Vg[:, mt, :, Dh:Dh + 1], 1.0)

            def vproj(mts):
                for mt in mts:
                    psA = acc.tile([128, 512], f32, tag="acc")
                    psB = acc.tile([128, 512], f32, tag="acc")
                    for kc in range(KC):
                        nc.tensor.matmul(
                            psA[:, 0:512], XT[:, kc, mt * 128:(mt + 1) * 128],
                            W1bf[:, kc, 1536:2048],
                            start=(kc == 0), stop=(kc == KC - 1))
                        nc.tensor.matmul(
                            psB[:, 0:256], XT[:, kc, mt * 128:(mt + 1) * 128],
                            W1bf[:, kc, 2048:2304],
                            start=(kc == 0), stop=(kc == KC - 1))
                    nc.vector.tensor_add(
                        Vg[:, mt, 0:8, 0:Dh],
                        psA[:].rearrange("p (h d) -> p h d", d=Dh),
                        b1vb[:, 0:512].rearrange("p (h d) -> p h d", d=Dh))
                    nc.vector.tensor_add(
                        Vg[:, mt, 8:12, 0:Dh],
                        psB[:, 0:256].rearrange("p (h d) -> p h d", d=Dh),
                        b1vb[:, 512:768].rearrange("p (h d) -> p h d", d=Dh))

            # ---- attention ----
            ZT = persist.tile([128, NPAIR, T], bf, tag="ZT")

            def scores_kt(hp, kt, pts):
                L = T - kt * 128
                ptile = ptpool.tile([128, 2, 1024], bf, tag="pt")
                for c_off in range(0, L, 512):
                    n = min(512, L - c_off)
                    sc = att.tile([128, 2, 512], f32, tag="att")
                    for h01 in range(2):
                        base = h01 * 64
                        nc.tensor.matmul(
                            sc[:, h01, 0:n],
                            QK[base:base + 64, 6 + hp, kt * 128:(kt + 1) * 128],
                            QK[base:base + 64, hp, kt * 128 + c_off:kt * 128 + c_off + n],
                            start=True, stop=True)
                    nc.scalar.activation(
                        ptile[:, :, c_off:c_off + n], sc[:, :, 0:n], EXP, scale=SCALE)
                # causal mask on the diagonal 128x128 block, both heads in one op
                nc.vector.tensor_mul(ptile[:, :, 0:128], ptile[:, :, 0:128], diagmask[:])
                pts.append(ptile)

            def av_mm(hp, c, pts):
                z = att.tile([128, 2, 512], f32, tag="att")
                kts = list(range(0, min(MT, 4 * (c + 1))))
                for kt in kts:
                    zoff = max(kt * 128 - c * 512, 0)
                    n = 512 - zoff
                    poff = max(c * 512 - kt * 128, 0)
                    for h01 in range(2):
                        nc.tensor.matmul(
                            z[0:Dh + 1, h01, zoff:zoff + n],
                            Vg[:, kt, 2 * hp + h01, 0:Dh + 1],
                            pts[kt][:, h01, poff:poff + n],
                            start=(kt == kts[0]), stop=(kt == kts[-1]),
                            skip_group_check=True)
                # single copy (incl. the bf16 denominator row) frees the PSUM
                # slot early; normalization is emitted later (av_norm) so
                # ScalarE keeps prioritizing the exp stream.
                zsb = zsbpool.tile([128, 2, 512], bf, tag="zsb")
                nc.vector.tensor_copy(zsb[0:Dh + 1, :, :], z[0:Dh + 1, :, :])
                return zsb

            def av_norm(hp, c, zsb, halves=1):
                # halves=2 pipelines the chain in 256-col pieces for the
                # latency-critical last pair
                w = 512 // halves
                for hf in range(halves):
                    lo, hi = hf * w, (hf + 1) * w
                    dln = dlnpool.tile([1, 2, 512], f32, tag="dln")
                    nc.scalar.activation(dln[:, :, 0:w], zsb[Dh:Dh + 1, :, lo:hi], LN)
                    rec = recpool.tile([1, 2, 512], bf, tag="rec")
                    nc.scalar.activation(rec[:, :, 0:w], dln[:, :, 0:w], EXP, scale=-1.0)
                    bc = bcpool.tile([64, 2, 512], bf, tag="bc")
                    nc.gpsimd.partition_broadcast(bc[:, :, 0:w], rec[:, :, 0:w])
                    for h01 in range(2):
                        nc.vector.tensor_mul(
                            ZT[h01 * 64:(h01 + 1) * 64, hp, c * 512 + lo:c * 512 + hi],
                            zsb[0:Dh, h01, lo:hi], bc[:, h01, 0:w])

            # ---- output projection, split-K: pairs 0-2 accumulated early,
            # pairs 3-5 + combine at the tail ----
            outA = persist.tile([128, MT, E], bf, tag="outA")

            def outproj_a(mts):
                # pairs 0..3: runs as late-phase PE filler once ZT 0-3 are done
                for mt in mts:
                    psA = acc.tile([128, 512], f32, tag="acc")
                    psB = acc.tile([128, 512], f32, tag="acc")
                    for pc in range(4):
                        nc.tensor.matmul(
                            psA[:], ZT[:, pc, mt * 128:(mt + 1) * 128],
                            W2bf[:, pc, 0:512], start=(pc == 0), stop=(pc == 3))
                        nc.tensor.matmul(
                            psB[:, 0:256], ZT[:, pc, mt * 128:(mt + 1) * 128],
                            W2bf[:, pc, 512:768], start=(pc == 0), stop=(pc == 3))
                    nc.vector.tensor_add(outA[:, mt, 0:512], psA[:], b2b[:, 0:512])
                    nc.vector.tensor_add(outA[:, mt, 512:768], psB[:, 0:256], b2b[:, 512:768])

            def outproj_b(mts):
                # pairs 4..5 tail; the outA partial (which already carries b2)
                # is accumulated via an identity matmul so the eviction is a
                # plain copy, split ScalarE/DVE (ScalarE is idle in the tail)
                for mt in mts:
                    osb = opool.tile([128, E], f32, tag="osb")
                    psA = acc.tile([128, 512], f32, tag="acc")
                    psB = acc.tile([128, 512], f32, tag="acc")
                    for pc in range(4, KC):
                        nc.tensor.matmul(
                            psA[:], ZT[:, pc, mt * 128:(mt + 1) * 128],
                            W2bf[:, pc, 0:512], start=(pc == 4), stop=False)
                        nc.tensor.matmul(
                            psB[:, 0:256], ZT[:, pc, mt * 128:(mt + 1) * 128],
                            W2bf[:, pc, 512:768], start=(pc == 4), stop=False)
                    nc.tensor.matmul(
                        psA[:], ident[:], outA[:, mt, 0:512],
                        start=False, stop=True)
                    nc.tensor.matmul(
                        psB[:, 0:256], ident[:], outA[:, mt, 512:768],
                        start=False, stop=True)
                    nc.scalar.activation(osb[:, 0:512], psA[:], COPY)
                    nc.vector.tensor_copy(osb[:, 512:768], psB[:, 0:256])
                    nc.sync.dma_start(
                        out=out_ext[mt * 128:(mt + 1) * 128, 0:512], in_=osb[:, 0:512])
                    nc.sync.dma_start(
                        out=out_ext[mt * 128:(mt + 1) * 128, 512:768], in_=osb[:, 512:768])

            # Software-pipelined pair loop: pair hp+1's first score batch is
            # emitted inside pair hp's body (before av_norm(hp,1)) so the next
            # exps queue on ScalarE ahead of the non-critical normalization
            # work instead of behind it. qk tiles are computed two pairs ahead.
            qk_mtile(0)
            qk_mtile(6)
            qk_mtile(1)
            qk_mtile(7)

            pts_cur = []
            for kt in range(4):
                scores_kt(0, kt, pts_cur)

            for hp in range(NPAIR):
                if hp == 0:
                    vproj(range(MT))
                zsb0 = av_mm(hp, 0, pts_cur)
                for kt in range(4, MT):
                    scores_kt(hp, kt, pts_cur)
                av_norm(hp, 0, zsb0)
                if hp + 2 < NPAIR:
                    qk_mtile(hp + 2)
                    qk_mtile(6 + hp + 2)
                zsb1 = av_mm(hp, 1, pts_cur)
                if hp == 1:
                    w2_dma()
                pts_next = []
                if hp + 1 < NPAIR:
                    for kt in range(4):
                        scores_kt(hp + 1, kt, pts_next)
                av_norm(hp, 1, zsb1, halves=(2 if hp == NPAIR - 1 else 1))
                if hp == 4:
                    outproj_a(range(0, 4))
                pts_cur = pts_next

            outproj_a(range(4, MT))
            outproj_b(range(0, 4))
            outproj_b(range(4, MT))

    nc.compile()
    return nc


def _get_nc():
    global _NC_CACHE
    if _NC_CACHE is None:
        _NC_CACHE = build_nc()
    return _NC_CACHE


def _in_maps(X, W1, b1, W2, b2):
    bfdt = ml_dtypes.bfloat16
    X = np.asarray(X, dtype=np.float32)
    assert X.shape == (B, T, E)
    W1b = np.ascontiguousarray(np.asarray(W1, dtype=np.float32).astype(bfdt))
    W2b = np.ascontiguousarray(np.asarray(W2, dtype=np.float32).astype(bfdt))
    b1 = np.ascontiguousarray(np.asarray(b1, dtype=np.float32))
    b2 = np.ascontiguousarray(np.asarray(b2, dtype=np.float32))
    XTs = [np.ascontiguousarray(X[i].T.astype(bfdt)) for i in range(B)]
    return [
        {"XT": XTs[i], "W1": W1b, "b1": b1, "W2": W2b, "b2": b2}
        for i in range(B)
    ]


def kernel(X, W1, b1, W2, b2):
    from concourse.bass_utils import run_bass_kernel_spmd

    nc = _get_nc()
    res = run_bass_kernel_spmd(nc, _in_maps(X, W1, b1, W2, b2), core_ids=list(range(B)))
    return np.stack([res.results[i]["out"] for i in range(B)]).astype(np.float32)


def kernel_traced(X, W1, b1, W2, b2, tmpdir=None):
    """Like kernel() but with neuron-profile tracing; returns (out, BassKernelResults)."""
    from concourse.bass_utils import run_bass_kernel_spmd

    nc = _get_nc()
    res = run_bass_kernel_spmd(
        nc, _in_maps(X, W1, b1, W2, b2), core_ids=list(range(B)),
        trace=True, tmpdir=tmpdir,
    )
    out = np.stack([res.results[i]["out"] for i in range(B)]).astype(np.float32)
    return out, res


# revision 51
# speedup vs baseline: 1.0523x; 1.0523x over previous
"""Trainium2 Bass kernel: causal multi-head self-attention block (B=8, T=1024, E=768, H=12).

Sharding: data-parallel over batch - one batch element per NeuronCore, 8 cores,
no collectives. Each core computes the full attention block for its batch row.

v2: host pre-transposes X and pre-casts X/W1/W2 to bf16 (no on-device casts or
PE transposes), drops the K-projection bias (softmax-invariant), single rotating
PSUM pool for scores+AV, interleaved accumulation chains for PSUM bank
alternation, batched ln/exp reciprocal + DMA partition-broadcast for the
softmax denominator (no GpSimd broadcast on the critical path).

Self-contained: hardcodes all shapes; only imports concourse (installed
system-wide) and ml_dtypes.
"""

import numpy as np
import ml_dtypes

B, T, E, H, Dh = 8, 1024, 768, 12, 64
F3 = 3 * E            # 2304
KC = E // 128         # 6 e-chunks
MT = T // 128         # 8 t-tiles
NPAIR = H // 2        # 6 head pairs
SCALE = 1.0 / float(np.sqrt(Dh))

_NC_CACHE = None


def build_nc():
    import concourse.mybir as mybir
    from concourse import bacc
    from concourse.tile import TileContext
    from concourse.masks import make_identity

    bf = mybir.dt.bfloat16
    f32 = mybir.dt.float32
    EXP = mybir.ActivationFunctionType.Exp
    LN = mybir.ActivationFunctionType.Ln
    COPY = mybir.ActivationFunctionType.Copy
    IDENT = mybir.ActivationFunctionType.Identity
    ACT_SET_LN_EXP = 6  # natural_log_exp_and_others: holds both Ln and Exp

    nc = bacc.Bacc("TRN2", target_bir_lowering=False, debug=False, num_devices=B, name="attn_dp2")

    XT_ext = nc.declare_dram_parameter("XT", [E, T], bf, isOutput=False)
    W1_ext = nc.declare_dram_parameter("W1", [E, F3], bf, isOutput=False)
    b1_ext = nc.declare_dram_parameter("b1", [F3], f32, isOutput=False)
    W2_ext = nc.declare_dram_parameter("W2", [E, E], bf, isOutput=False)
    b2_ext = nc.declare_dram_parameter("b2", [E], f32, isOutput=False)
    out_ext = nc.declare_dram_parameter("out", [T, E], f32, isOutput=True)

    with TileContext(nc) as tc:
        with (
            tc.tile_pool(name="persist", bufs=1) as persist,
            tc.tile_pool(name="stage", bufs=2) as stage,
            tc.tile_pool(name="ptpool", bufs=13) as ptpool,
            tc.tile_pool(name="zsbpool", bufs=3) as zsbpool,
            tc.tile_pool(name="dlnpool", bufs=2) as dlnpool,
            tc.tile_pool(name="recpool", bufs=2) as recpool,
            tc.tile_pool(name="bcpool", bufs=3) as bcpool,
            tc.tile_pool(name="opool", bufs=2) as opool,
            tc.tile_pool(name="att", bufs=3, space="PSUM") as att,
            tc.tile_pool(name="acc", bufs=2, space="PSUM") as acc,
        ):
            # ---- input DMAs first: the whole kernel is gated on W1/XT landing.
            # Interleaved per kc so the first qk chains can start as soon as the
            # first chunks arrive; W2 is DMA'd later (emitted inside the hp
            # loop) since it is only needed for the output projection.
            XT = persist.tile([128, KC, T], bf, tag="XT")
            W1bf = persist.tile([128, KC, F3], bf, tag="W1bf")
            W2bf = persist.tile([128, KC, E], bf, tag="W2bf")
            # Q/K weight columns + XT first (gate the first qk chains); V
            # columns of W1 are only needed by vproj, which runs later.
            for kc in range(KC):
                nc.sync.dma_start(out=XT[:, kc, :], in_=XT_ext[kc * 128:(kc + 1) * 128, :])
                nc.sync.dma_start(out=W1bf[:, kc, 0:1536], in_=W1_ext[kc * 128:(kc + 1) * 128, 0:1536])
            for kc in range(KC):
                nc.sync.dma_start(out=W1bf[:, kc, 1536:2304], in_=W1_ext[kc * 128:(kc + 1) * 128, 1536:2304])

            def w2_dma():
                for kc in range(KC):
                    nc.sync.dma_start(out=W2bf[:, kc, :], in_=W2_ext[kc * 128:(kc + 1) * 128, :])

            # One activation-table load for the whole kernel (covers Exp + Ln).
            nc.scalar.add_instruction(mybir.InstLoadActFuncSet(
                name=nc.get_next_instruction_name(), ins=[], outs=[],
                act_func_set_id=ACT_SET_LN_EXP))

            # identity for folding the outA partial into outproj_b's PE chain
            ident = persist.tile([128, 128], bf, tag="ident")
            make_identity(nc, ident[:])

            # ---- constants ----
            # multiplicative causal mask for the diagonal 128x128 block, for
            # both heads of a pair: mask[k, h01, q] = 1 where q >= k else 0
            diagmask = persist.tile([128, 2, 128], bf, tag="diagmask")
            nc.gpsimd.memset(diagmask[:], 1.0)
            for h01 in range(2):
                nc.gpsimd.affine_select(
                    out=diagmask[:, h01, :], in_=diagmask[:, h01, :],
                    compare_op=mybir.AluOpType.is_ge, fill=0.0, base=0,
                    pattern=[[1, 128]], channel_multiplier=-1,
                )

            # per-partition bias for the Q part of b1: b1qk[p, m] = b1[m*128 + p]
            # (K bias dropped: it shifts every score row by a per-query constant,
            # which softmax is invariant to.)
            b1qk = persist.tile([128, 6], f32, tag="b1qk")
            nc.sync.dma_start(
                out=b1qk[:], in_=b1_ext[0:768].rearrange("(m p) -> p m", p=128)
            )
            # row biases, pre-broadcast across partitions
            b1v_f = stage.tile([1, E], f32, tag="rowstage")
            nc.sync.dma_start(out=b1v_f[:], in_=b1_ext[None, 1536:2304])
            b1vb = persist.tile([128, E], f32, tag="b1vb")
            nc.gpsimd.partition_broadcast(b1vb[:], b1v_f[:])
            b2_f = stage.tile([1, E], f32, tag="rowstage")
            nc.sync.dma_start(out=b2_f[:], in_=b2_ext[None, :])
            b2b = persist.tile([128, E], f32, tag="b2b")
            nc.gpsimd.partition_broadcast(b2b[:], b2_f[:])

            # QK[p, m, t]: m 0..5 = Q^T blocks (f rows m*128..), m 6..11 = K^T blocks
            QK = persist.tile([128, 12, T], bf, tag="QK")

            def qk_mtile(m):
                # two 512-column chains, kc-interleaved so consecutive PE
                # writes alternate PSUM banks
                psA = acc.tile([128, 512], f32, tag="acc")
                psB = acc.tile([128, 512], f32, tag="acc")
                for kc in range(KC):
                    nc.tensor.matmul(
                        psA[:], W1bf[:, kc, m * 128:(m + 1) * 128],
                        XT[:, kc, 0:512], start=(kc == 0), stop=(kc == KC - 1))
                    nc.tensor.matmul(
                        psB[:], W1bf[:, kc, m * 128:(m + 1) * 128],
                        XT[:, kc, 512:1024], start=(kc == 0), stop=(kc == KC - 1))
                # evictions on DVE: ScalarE is the critical engine (exp) and
                # must not queue eviction work ahead of the exp stream
                if m < 6:
                    nc.vector.tensor_scalar_add(QK[:, m, 0:512], psA[:], b1qk[:, m:m + 1])
                    nc.vector.tensor_scalar_add(QK[:, m, 512:1024], psB[:], b1qk[:, m:m + 1])
                else:
                    nc.vector.tensor_copy(QK[:, m, 0:512], psA[:])
                    nc.vector.tensor_copy(QK[:, m, 512:1024], psB[:])

            # ---- V projection into V_aug[t-part, kt, h, 0:64] with ones col at 64 ----
            Vg = persist.tile([128, MT, H, Dh + 1], bf, tag="Vg")
            for mt in range(MT):
                nc.gpsimd.memset(Vg[:, mt, :, Dh:Dh + 1], 1.0)

            def vproj(mts):
                for mt in mts:
                    psA = acc.tile([128, 512], f32, tag="acc")
                    psB = acc.tile([128, 512], f32, tag="acc")
                    for kc in range(KC):
                        nc.tensor.matmul(
                            psA[:, 0:512], XT[:, kc, mt * 128:(mt + 1) * 128],
                            W1bf[:, kc, 1536:2048],
                            start=(kc == 0), stop=(kc == KC - 1))
                        nc.tensor.matmul(
                            psB[:, 0:256], XT[:, kc, mt * 128:(mt + 1) * 128],
                            W1bf[:, kc, 2048:2304],
                            start=(kc == 0), stop=(kc == KC - 1))
                    nc.vector.tensor_add(
                        Vg[:, mt, 0:8, 0:Dh],
                        psA[:].rearrange("p (h d) -> p h d", d=Dh),
                        b1vb[:, 0:512].rearrange("p (h d) -> p h d", d=Dh))
                    nc.vector.tensor_add(
                        Vg[:, mt, 8:12, 0:Dh],
                        psB[:, 0:256].rearrange("p (h d) -> p h d", d=Dh),
                        b1vb[:, 512:768].rearrange("p (h d) -> p h d", d=Dh))

            # ---- attention ----
            ZT = persist.tile([128, NPAIR, T], bf, tag="ZT")

            def scores_kt(hp, kt, pts):
                L = T - kt * 128
                ptile = ptpool.tile([128, 2, 1024], bf, tag="pt")
                for c_off in range(0, L, 512):
                    n = min(512, L - c_off)
                    sc = att.tile([128, 2, 512], f32, tag="att")
                    for h01 in range(2):
                        base = h01 * 64
                        nc.tensor.matmul(
                            sc[:, h01, 0:n],
                            QK[base:base + 64, 6 + hp, kt * 128:(kt + 1) * 128],
                            QK[base:base + 64, hp, kt * 128 + c_off:kt * 128 + c_off + n],
                            start=True, stop=True)
                    nc.scalar.activation(
                        ptile[:, :, c_off:c_off + n], sc[:, :, 0:n], EXP, scale=SCALE)
                # causal mask on the diagonal 128x128 block, both heads in one op
                nc.vector.tensor_mul(ptile[:, :, 0:128], ptile[:, :, 0:128], diagmask[:])
                pts.append(ptile)

            def av_mm(hp, c, pts):
                z = att.tile([128, 2, 512], f32, tag="att")
                kts = list(range(0, min(MT, 4 * (c + 1))))
                for kt in kts:
                    zoff = max(kt * 128 - c * 512, 0)
                    n = 512 - zoff
                    poff = max(c * 512 - kt * 128, 0)
                    for h01 in range(2):
                        nc.tensor.matmul(
                            z[0:Dh + 1, h01, zoff:zoff + n],
                            Vg[:, kt, 2 * hp + h01, 0:Dh + 1],
                            pts[kt][:, h01, poff:poff + n],
                            start=(kt == kts[0]), stop=(kt == kts[-1]),
                            skip_group_check=True)
                # single copy (incl. the bf16 denominator row) frees the PSUM
                # slot early; normalization is emitted later (av_norm) so
                # ScalarE keeps prioritizing the exp stream.
                zsb = zsbpool.tile([128, 2, 512], bf, tag="zsb")
                nc.vector.tensor_copy(zsb[0:Dh + 1, :, :], z[0:Dh + 1, :, :])
                return zsb

            def av_norm(hp, c, zsb, halves=1):
                # halves=2 pipelines the chain in 256-col pieces for the
                # latency-critical last pair
                w = 512 // halves
                for hf in range(halves):
                    lo, hi = hf * w, (hf + 1) * w
                    dln = dlnpool.tile([1, 2, 512], f32, tag="dln")
                    nc.scalar.activation(dln[:, :, 0:w], zsb[Dh:Dh + 1, :, lo:hi], LN)
                    rec = recpool.tile([1, 2, 512], bf, tag="rec")
                    nc.scalar.activation(rec[:, :, 0:w], dln[:, :, 0:w], EXP, scale=-1.0)
                    bc = bcpool.tile([64, 2, 512], bf, tag="bc")
                    nc.gpsimd.partition_broadcast(bc[:, :, 0:w], rec[:, :, 0:w])
                    for h01 in range(2):
                        nc.vector.tensor_mul(
                            ZT[h01 * 64:(h01 + 1) * 64, hp, c * 512 + lo:c * 512 + hi],
                            zsb[0:Dh, h01, lo:hi], bc[:, h01, 0:w])

            # ---- output projection, split-K: pairs 0-2 accumulated early,
            # pairs 3-5 + combine at the tail ----
            outA = persist.tile([128, MT, E], bf, tag="outA")

            def outproj_a(mts):
                # pairs 0..3: runs as late-phase PE filler once ZT 0-3 are done
                for mt in mts:
                    psA = acc.tile([128, 512], f32, tag="acc")
                    psB = acc.tile([128, 512], f32, tag="acc")
                    for pc in range(4):
                        nc.tensor.matmul(
                            psA[:], ZT[:, pc, mt * 128:(mt + 1) * 128],
                            W2bf[:, pc, 0:512], start=(pc == 0), stop=(pc == 3))
                        nc.tensor.matmul(
                            psB[:, 0:256], ZT[:, pc, mt * 128:(mt + 1) * 128],
                            W2bf[:, pc, 512:768], start=(pc == 0), stop=(pc == 3))
                    nc.vector.tensor_add(outA[:, mt, 0:512], psA[:], b2b[:, 0:512])
                    nc.vector.tensor_add(outA[:, mt, 512:768], psB[:, 0:256], b2b[:, 512:768])

            def outproj_b(mts):
                # pairs 4..5 tail; the outA partial (which already carries b2)
                # is accumulated via an identity matmul so the eviction is a
                # plain copy, split ScalarE/DVE (ScalarE is idle in the tail)
                for mt in mts:
                    osb = opool.tile([128, E], f32, tag="osb")
                    psA = acc.tile([128, 512], f32, tag="acc")
                    psB = acc.tile([128, 512], f32, tag="acc")
                    for pc in range(4, KC):
                        nc.tensor.matmul(
                            psA[:], ZT[:, pc, mt * 128:(mt + 1) * 128],
                            W2bf[:, pc, 0:512], start=(pc == 4), stop=False)
                        nc.tensor.matmul(
                            psB[:, 0:256], ZT[:, pc, mt * 128:(mt + 1) * 128],
                            W2bf[:, pc, 512:768], start=(pc == 4), stop=False)
                    nc.tensor.matmul(
                        psA[:], ident[:], outA[:, mt, 0:512],
                        start=False, stop=True)
                    nc.tensor.matmul(
                        psB[:, 0:256], ident[:], outA[:, mt, 512:768],
                        start=False, stop=True)
                    nc.scalar.activation(osb[:, 0:512], psA[:], COPY)
                    nc.vector.tensor_copy(osb[:, 512:768], psB[:, 0:256])
                    nc.sync.dma_start(
                        out=out_ext[mt * 128:(mt + 1) * 128, 0:512], in_=osb[:, 0:512])
                    nc.sync.dma_start(
                        out=out_ext[mt * 128:(mt + 1) * 128, 512:768], in_=osb[:, 512:768])

            # Software-pipelined pair loop: pair hp+1's first score batch is
            # emitted inside pair hp's body (before av_norm(hp,1)) so the next
            # exps queue on ScalarE ahead of the non-critical normalization
            # work instead of behind it. qk tiles are computed two pairs ahead.
            qk_mtile(0)
            qk_mtile(6)
            qk_mtile(1)
            qk_mtile(7)

            pts_cur = []
            for kt in range(4):
                scores_kt(0, kt, pts_cur)

            for hp in range(NPAIR):
                if hp == 0:
                    vproj(range(MT))
                zsb0 = av_mm(hp, 0, pts_cur)
                for kt in range(4, MT):
                    scores_kt(hp, kt, pts_cur)
                av_norm(hp, 0, zsb0)
                if hp + 2 < NPAIR:
                    qk_mtile(hp + 2)
                    qk_mtile(6 + hp + 2)
                zsb1 = av_mm(hp, 1, pts_cur)
                if hp == 1:
                    w2_dma()
                pts_next = []
                if hp + 1 < NPAIR:
                    for kt in range(4):
                        scores_kt(hp + 1, kt, pts_next)
                av_norm(hp, 1, zsb1, halves=(2 if hp == NPAIR - 1 else 1))
                if hp == 4:
                    outproj_a(range(0, 4))
                pts_cur = pts_next

            outproj_a(range(4, MT))
            outproj_b(range(0, 4))
            outproj_b(range(4, MT))

    nc.compile()
    return nc


def _get_nc():
    global _NC_CACHE
    if _NC_CACHE is None:
        _NC_CACHE = build_nc()
    return _NC_CACHE


def _in_maps(X, W1, b1, W2, b2):
    bfdt = ml_dtypes.bfloat16
    X = np.asarray(X, dtype=np.float32)
    assert X.shape == (B, T, E)
    W1b = np.ascontiguousarray(np.asarray(W1, dtype=np.float32).astype(bfdt))
    W2b = np.ascontiguousarray(np.asarray(W2, dtype=np.float32).astype(bfdt))
    b1 = np.ascontiguousarray(np.asarray(b1, dtype=np.float32))
    b2 = np.ascontiguousarray(np.asarray(b2, dtype=np.float32))
    XTs = [np.ascontiguousarray(X[i].T.astype(bfdt)) for i in range(B)]
    return [
        {"XT": XTs[i], "W1": W1b, "b1": b1, "W2": W2b, "b2": b2}
        for i in range(B)
    ]


def kernel(X, W1, b1, W2, b2):
    from concourse.bass_utils import run_bass_kernel_spmd

    nc = _get_nc()
    res = run_bass_kernel_spmd(nc, _in_maps(X, W1, b1, W2, b2), core_ids=list(range(B)))
    return np.stack([res.results[i]["out"] for i in range(B)]).astype(np.float32)


def kernel_traced(X, W1, b1, W2, b2, tmpdir=None):
    """Like kernel() but with neuron-profile tracing; returns (out, BassKernelResults)."""
    from concourse.bass_utils import run_bass_kernel_spmd

    nc = _get_nc()
    res = run_bass_kernel_spmd(
        nc, _in_maps(X, W1, b1, W2, b2), core_ids=list(range(B)),
        trace=True, tmpdir=tmpdir,
    )
    out = np.stack([res.results[i]["out"] for i in range(B)]).astype(np.float32)
    return out, res
